# revision 59
# baseline (speedup 1.0000x reference)
"""AlignBlock kernel for 8 TRN2 NeuronCores.

Reference computation (B=2, C=2, T=500, F=129, H=16, D=100):
  Q = conv1x1(x_mic; w_mic, b_mic)        (B,H,T,F)
  K = conv1x1(x_ref; w_ref, b_ref)        (B,H,T,F)
  V[b,h,t,d]  = sum_f Q[b,h,t,f] * Kpad[b,h,t-99+d,f]       (delay window)
  V2 = conv2d(V, w_conv (1,H,5,3), causal-T pad (4,0), d pad (1,1)) + b_conv
  A  = softmax_d(V2[:,0])                 (B,T,D)
  y[b,c,t,f] = sum_d x_refpad[b,c,t-99+d,f] * A[b,t,d]

Key algebraic restructuring (all exact):
  - The H dimension is folded on the host: with augmented channels
    xm~ = [xm0, xm1, u], xr~ = [xr0, xr1, v] (u/v = validity masks emulating
    the reference's zero padding of Q rows / K columns), the conv input
    planes are sum_h w_conv[h]*V[h] = sum_{q=(cm,cr)} Wc[q] * XC[q] where
    XC[q][t,t'] = sum_f xm~[cm,t,f] xr~[cr,t',f]  (9 raw correlation planes)
    and Wc[q,i,j] = sum_h w_conv[h,i,j] wm~[h,cm] wr~[h,cr].
  - The causal 5-tap T conv becomes banded-matrix matmuls (contraction over
    conv input rows); the 3 d-taps are free-dim shifted column reads.
  - softmax(V2 + b_conv) == softmax(V2): b_conv is dropped.
  - y is a matmul contracting t' with the banded attention matrix A_band.

Sharding: sequence-parallel over T, 63 output frames/core (T padded 500->504),
each core loads its input slice with halos host-side; no collectives.

DMA scheduling (measured on TRN2 DGE):
  - strided-DRAM-side DMAs pace ~27-45ns/descriptor serially per queue;
    contiguous flat <=2KB descriptors flow concurrently (~3-13ns each).
  - per-queue sustained throughput caps around 25-60GB/s; the band writes
    and diagonal regathers are row-split/spread across the sync HWDGE,
    scalar HWDGE and gpsimd SWDGE queues.
  - softmax is max-free (logits ~ +-17), normalization is folded into the
    final psum read-out, and the attention skew write is one flat
    aggregating row write per batch feeding a single xbar transpose pair.
"""

import os
import sys

import numpy as np

sys.path.insert(0, "/opt/trn_rl_repo")

# ---- problem constants (hardcoded per the staged problem) ----
B, C, T, F = 2, 2, 500, 129
H, D = 16, 100
NCORES = 8
TL = 63               # output frames per core
TP = NCORES * TL      # padded T = 504
R = TL + 4            # conv input rows per core (67)
TH = TL + D + 3       # x_ref halo columns per core (166)
NQ = 9                # augmented channel pairs
DW = D + 2            # padded delay width incl. zero edge cols (102)

_CACHE = {}


def _np_reference(x_mic, x_ref, w_mic, b_mic, w_ref, b_ref, w_conv, b_conv, delay):
    """Pure-numpy fallback, exact mirror of the jax reference."""
    Bn, Cn, Tn, Fn = x_mic.shape
    Dn = int(delay)
    Q = np.einsum("bctf,hc->bhtf", x_mic, w_mic) + b_mic[None, :, None, None]
    K = np.einsum("bctf,hc->bhtf", x_ref, w_ref) + b_ref[None, :, None, None]
    idx = np.arange(Tn)[:, None] + np.arange(Dn)[None, :]
    Kp = np.pad(K, ((0, 0), (0, 0), (Dn - 1, 0), (0, 0)))
    Ku = Kp[:, :, idx, :]
    V = np.einsum("bhtf,bhtdf->bhtd", Q, Ku)
    Hh = w_conv.shape[1]
    Vp = np.pad(V, ((0, 0), (0, 0), (4, 0), (1, 1)))
    out = np.zeros((Bn, Tn, Dn), np.float32)
    for i in range(5):
        for j in range(3):
            out += np.einsum(
                "bhtd,h->btd", Vp[:, :, i : i + Tn, j : j + Dn], w_conv[0, :, i, j]
            )
    out += b_conv[0]
    m = out.max(-1, keepdims=True)
    e = np.exp(out - m)
    A = e / e.sum(-1, keepdims=True)
    Rp = np.pad(x_ref, ((0, 0), (0, 0), (Dn - 1, 0), (0, 0)))
    Ru = Rp[:, :, idx, :]
    return np.einsum("bctdf,btd->bctf", Ru, A).astype(np.float32)


def _build_graph():
    """Build + compile the single-core SPMD Bass graph (identical on all cores)."""
    from concourse import bacc, mybir, tile

    dt = mybir.dt
    f32 = dt.float32
    bf16 = dt.bfloat16

    nc = bacc.Bacc(
        "TRN2", target_bir_lowering=False, debug=False, num_devices=NCORES
    )

    # external I/O (per-core shards, host-prepared layouts)
    xmt = nc.dram_tensor("xmt", [F, B, 3, R], bf16, kind="ExternalInput")
    xrt = nc.dram_tensor("xrt", [F, B, 3, TH], bf16, kind="ExternalInput")
    xrn = nc.dram_tensor("xrn", [TH, B, C, F], bf16, kind="ExternalInput")
    # banded conv weights + 6 trailing cols holding the f=128 row of xmt as
    # per-partition scalars for the fused rank-1 correlation correction
    NBW = NQ * 3 * TL
    bcv = nc.dram_tensor("bcv", [R, NBW + 6], bf16, kind="ExternalInput")
    out = nc.dram_tensor("out", [B, C, TL, F], f32, kind="ExternalOutput")

    # per-b correlation scratch: 3 cm-planes [g][r][t'*3] (q-triplets
    # interleaved stride-3 within each plane)
    GQ = 3
    sxc = [nc.dram_tensor(f"sxc{b}", [3 * R * TH * GQ + 64], bf16) for b in range(B)]
    # row-banded attention scratch, one tensor per batch (DRAM dependency
    # tracking is tensor-granular; separate tensors let b0's xbar regather
    # start before b1's softmax lands)
    SGS = 64 * 256
    sg = [nc.dram_tensor(f"sg{b}", [SGS + 64], bf16) for b in range(B)]

    VecI64Pair = None

    def strided_ap(handle, offset_el, dims):
        """AP on a flat DRAM tensor with explicit [stride, size] dims."""
        nonlocal VecI64Pair
        a = handle.ap().copy()
        if VecI64Pair is None:
            VecI64Pair = type(a.ap)
        a.ap = VecI64Pair([list(d) for d in dims])
        a.offset = offset_el
        return a

    with tile.TileContext(nc) as tc:
        with (
            tc.tile_pool(name="w", bufs=1) as wp,
            tc.tile_pool(name="xcps", bufs=3, space="PSUM") as xcp,
            tc.tile_pool(name="cvps", bufs=1, space="PSUM") as cvp,
            tc.tile_pool(name="vps", bufs=2, space="PSUM") as vp,
            tc.tile_pool(name="yps", bufs=2, space="PSUM") as yp,
            tc.tile_pool(name="st", bufs=6) as sp,
            tc.tile_pool(name="sm", bufs=2) as smp,
        ):
            # ---- persistent input tiles ----
            xmt0 = wp.tile([128, B, 3, R], bf16, tag="xmt0")
            xmt1 = wp.tile([1, B, 3, R], bf16, tag="xmt1")
            xrt0 = wp.tile([128, B, 3, TH], bf16, tag="xrt0")
            xrt1 = wp.tile([1, B, 3, TH], bf16, tag="xrt1")
            xrn0 = wp.tile([128, B, C, F], bf16, tag="xrn0")
            xrn1 = wp.tile([TH - 128, B, C, F], bf16, tag="xrn1")
            bcw = wp.tile([R, NBW + 6], bf16, tag="bcw")

            # XC-critical tensors first; big flowing loads on sync whose queue
            # the band writes now avoid, tinies + halves on scalar
            nc.sync.dma_start(out=xrt0[:], in_=xrt[0:128])
            nc.scalar.dma_start(out=xrt1[:], in_=xrt[128:129])
            nc.sync.dma_start(out=xmt0[:], in_=xmt[0:128])
            nc.scalar.dma_start(out=xmt1[:], in_=xmt[128:129])
            nc.sync.dma_start(out=bcw[0:34], in_=bcv[0:34])
            nc.scalar.dma_start(out=bcw[34:R], in_=bcv[34:R])
            nc.sync.dma_start(out=xrn0[:], in_=xrn[0:128])
            nc.scalar.dma_start(out=xrn1[:], in_=xrn[128:TH])

            # PE warm-up: dense dummy matmuls during the DMA load prologue to
            # release the HAM clock gate before real matmuls.
            wsrc = wp.tile([128, 512], bf16, tag="wsrc")
            nc.gpsimd.memset(wsrc[:], 0.0)
            wps = cvp.tile([128, 512], f32, tag="wps")
            for _ in range(6):
                nc.tensor.matmul(
                    out=wps[:], lhsT=wsrc[:, 0:128], rhs=wsrc[:],
                    start=True, stop=True,
                )

            # attention tile [128, 257]: b blocks at 64-partition alignment,
            # exp written to cols [4:104), rest stays zero; full 257-wide rows
            # written flat to sg give the banded (skewed) layout with
            # aggregating descriptors.
            att = wp.tile([128, 257], bf16, tag="att")
            nc.gpsimd.memset(att[:], 0.0)
            # zero the sg tail beyond the last written row in each b block
            ztail = wp.tile([2, SGS - TL * 257], bf16, tag="ztail")
            nc.gpsimd.memset(ztail[:], 0.0)
            nc.gpsimd.dma_start(
                out=strided_ap(sg[0], TL * 257, [[SGS, 1], [1, SGS - TL * 257]]),
                in_=ztail[0:1, :],
            )
            nc.gpsimd.dma_start(
                out=strided_ap(sg[1], TL * 257, [[SGS, 1], [1, SGS - TL * 257]]),
                in_=ztail[1:2, :],
            )

            # rect correlation planes: [r, b, g, dw, qs], edge cols stay zero
            xcd = wp.tile([R, B, 3, DW, GQ], bf16, tag="xcd")
            nc.gpsimd.memset(xcd[:], 0.0)

            # f=128 row of xrt replicated to all partitions: the 129th
            # contraction row becomes a fused DVE rank-1 update instead of a
            # K=1 matmul (which costs as much as the K=128 one)
            xr1rep = wp.tile([R, B, 3, TH], bf16, tag="xr1rep")
            nc.gpsimd.partition_broadcast(xr1rep[:], xrt1[:])

            # ---- stage 1: XC matmuls -> psum read-out (vector fuses the
            # rank-1 f=128 term; activation engine copies for 2 planes whose
            # K=1 matmul stays) -> band writes ----
            for b in range(B):
                for g in range(3):
                    xsb = sp.tile([R, TH, GQ], bf16, tag="xsb")
                    on_vec = (b, g) in ((0, 0), (0, 1), (0, 2), (1, 2))
                    # one batched matmul computes the 3 (cm, cr) planes
                    pxc = xcp.tile([R, GQ, TH], f32, tag="pxc")
                    nc.tensor.matmul(
                        out=pxc[:], lhsT=xmt0[:, b, g, :],
                        rhs=xrt0[:, b, :, :], start=True, stop=on_vec,
                    )
                    if not on_vec:
                        nc.tensor.matmul(
                            out=pxc[:], lhsT=xmt1[:, b, g, :],
                            rhs=xrt1[:, b, :, :], start=False, stop=True,
                        )
                    xout = xsb[:].transpose([0, 2, 1])
                    if on_vec:
                        nc.vector.scalar_tensor_tensor(
                            out=xout, in0=xr1rep[:, b, :, :],
                            scalar=bcw[:, NBW + 3 * b + g : NBW + 3 * b + g + 1],
                            in1=pxc[:],
                            op0=mybir.AluOpType.mult, op1=mybir.AluOpType.add,
                        )
                    else:
                        nc.scalar.activation(
                            out=xout, in_=pxc[:],
                            func=mybir.ActivationFunctionType.Copy,
                        )
                    # contiguous flat DRAM dst; keep the paced sync queue free
                    # for regathers: scalar takes 45 rows, SWDGE the rest
                    base = g * R * TH * GQ
                    for weng, r0, r1 in (
                        (nc.scalar, 0, 45), (nc.gpsimd, 45, R),
                    ):
                        weng.dma_start(
                            out=strided_ap(
                                sxc[b], base + r0 * TH * GQ,
                                [[TH * GQ, r1 - r0], [1, TH * GQ]],
                            ),
                            in_=xsb[r0:r1],
                        )

            # diagonal regather xcd[r, b, g, 1+d, qs] = XC_q[r, r+d]; sync's
            # queue is idle after the loads, so it takes the b0 planes (and
            # one more) while scalar finishes the band writes
            rg_eng = {
                (0, 0): (nc.sync,), (0, 1): (nc.sync,), (0, 2): (nc.sync,),
                (1, 0): (nc.scalar,), (1, 1): (nc.sync, nc.scalar),
                (1, 2): (nc.scalar,),
            }
            for b in range(B):
                for g in range(3):
                    engs = rg_eng[(b, g)]
                    for i, eng in enumerate(engs):
                        r0 = 0 if i == 0 else 34
                        r1 = R if i == len(engs) - 1 else 34
                        eng.dma_start(
                            out=xcd[r0:r1, b, g, 1 : 1 + D, :],
                            in_=strided_ap(
                                sxc[b],
                                g * R * TH * GQ + r0 * GQ * (TH + 1),
                                [[GQ * (TH + 1), r1 - r0], [1, D * GQ]],
                            ),
                        )

            # ---- stage 2: folded conv as banded matmuls, split per batch so
            # b0's softmax/attention tail overlaps b1's conv ----
            n_mm = NQ * 3
            v2b = []
            for b in range(B):
                vb = vp.tile([TL, D], f32, tag="v2")
                k = 0
                for g in range(3):
                    for qs in range(GQ):
                        q = g * GQ + qs
                        for j in range(3):
                            kb = (q * 3 + j) * TL
                            nc.tensor.matmul(
                                out=vb[:],
                                lhsT=bcw[:, kb : kb + TL],
                                rhs=xcd[:, b, g, j : j + D, qs],
                                start=(k == 0), stop=(k == n_mm - 1),
                            )
                            k += 1
                v2b.append(vb)

            # ---- stages 3+4, pipelined per batch: softmax (max-free: logits
            # ~ +-17, exp fits fp32/bf16), banded skew write, xbar regather,
            # delay-weighted matmul with normalization folded into the psum
            # read-out, contiguous per-plane output write ----
            ssum = smp.tile([128, 1], f32, tag="ssum")
            rin = smp.tile([128, 1], f32, tag="rin")
            yout = wp.tile([TL, B, C, F], f32, tag="yout")
            for b in range(B):
                nc.scalar.activation(
                    out=att[64 * b : 64 * b + 63, 4:104], in_=v2b[b][:],
                    func=mybir.ActivationFunctionType.Exp,
                    accum_out=ssum[64 * b : 64 * b + 63, :],
                )
                nc.vector.reciprocal(
                    rin[64 * b : 64 * b + 63, :], ssum[64 * b : 64 * b + 63, :]
                )
                # unnormalized exp -> banded DRAM scratch (flat, aggregating)
                weng = nc.sync if b == 0 else nc.scalar
                weng.dma_start(
                    out=strided_ap(sg[b], 0, [[257, TL], [1, 257]]),
                    in_=att[64 * b : 64 * b + 63, :],
                )
                # per-batch xbar pair, both on one queue so neither instruction
                # gets parked behind the other batch's waits
                xeng = nc.sync if b == 0 else nc.scalar
                a0 = wp.tile([128, 64], bf16, tag=f"a0_{b}")
                xeng.dma_start_transpose(
                    out=a0[:],
                    in_=strided_ap(sg[b], 0, [[256, 64], [1, 128]]),
                )
                a1 = wp.tile([128, 64], bf16, tag=f"a1_{b}")
                xeng.dma_start_transpose(
                    out=a1[:],
                    in_=strided_ap(sg[b], 128, [[256, 64], [1, 128]]),
                )
                for c in range(C):
                    py = yp.tile([TL, F], f32, tag="py")
                    nc.tensor.matmul(
                        out=py[:], lhsT=a0[:, 0:TL], rhs=xrn0[:, b, c, :],
                        start=True, stop=False,
                    )
                    nc.tensor.matmul(
                        out=py[:], lhsT=a1[0 : TH - 128, 0:TL],
                        rhs=xrn1[:, b, c, :],
                        start=False, stop=True,
                    )
                    nc.vector.tensor_scalar_mul(
                        yout[:, b, c, :], py[:], rin[64 * b : 64 * b + 63, :]
                    )
                    eng = nc.sync if c == 0 else nc.scalar
                    eng.dma_start(
                        out=strided_ap(
                            out, (b * C + c) * TL * F, [[F, TL], [1, F]]
                        ),
                        in_=yout[:, b, c, :],
                    )

    nc.compile()
    return nc


def _prepare_inputs(x_mic, x_ref, w_mic, b_mic, w_ref, b_ref, w_conv):
    """Host-side sharding + weight folding. Returns in_maps (one dict/core)."""
    from ml_dtypes import bfloat16

    # padded arrays: xm rows [t0-4, t0+63), xr cols [t0-103, t0+63)
    xm_pad = np.zeros((B, C, 4 + TP, F), np.float32)
    xm_pad[:, :, 4 : 4 + T] = x_mic
    xr_pad = np.zeros((B, C, D + 3 + TP, F), np.float32)
    xr_pad[:, :, D + 3 : D + 3 + T] = x_ref

    # folded conv weights: Wc[cm, cr, i, j] = sum_h w_conv * wm~ * wr~
    wt = np.asarray(w_conv, np.float64)[0]          # (H, 5, 3)
    wtm = np.concatenate([w_mic, b_mic[:, None]], 1).astype(np.float64)  # (H,3)
    wtr = np.concatenate([w_ref, b_ref[:, None]], 1).astype(np.float64)  # (H,3)
    Wc = np.einsum("hij,hm,hr->mrij", wt, wtm, wtr)  # (3,3,5,3)

    # banded conv matrices bcv[r, q, j, tau] = Wc[q, r-tau, j]
    bcv = np.zeros((R, 3, 3, 3, TL), np.float32)
    for i in range(5):
        for j in range(3):
            bcv[np.arange(TL) + i, :, :, j, np.arange(TL)] = np.float32(
                Wc[:, :, i, j]
            )[None]
    bcv = bcv.reshape(R, NQ * 3 * TL).astype(bfloat16)

    in_maps = []
    for i in range(NCORES):
        t0 = i * TL
        xm_s = xm_pad[:, :, t0 : t0 + R]          # (B,C,R,F) rows t0-4..t0+62
        xr_s = xr_pad[:, :, t0 : t0 + TH]         # (B,C,TH,F) cols t0-103..t0+62
        u = (np.arange(R) + t0 - 4 >= 0).astype(np.float32)
        v = (np.arange(TH) + t0 - D - 3 >= 0).astype(np.float32)

        xmt = np.empty((B, 3, R, F), np.float32)
        xmt[:, :C] = xm_s
        xmt[:, C] = u[:, None]
        xmt = np.ascontiguousarray(xmt.transpose(3, 0, 1, 2)).astype(bfloat16)
        # f=128 row of xmt as per-partition scalars, appended to bcv cols
        xm1c = np.ascontiguousarray(xmt[128].transpose(2, 0, 1).reshape(R, 6))
        bcv_full = np.concatenate([bcv, xm1c], axis=1)

        xrt = np.empty((B, 3, TH, F), np.float32)
        xrt[:, :C] = xr_s
        xrt[:, C] = v[:, None]
        xrt = np.ascontiguousarray(xrt.transpose(3, 0, 1, 2)).astype(bfloat16)

        xrn = np.ascontiguousarray(xr_s.transpose(2, 0, 1, 3)).astype(bfloat16)

        in_maps.append({"xmt": xmt, "xrt": xrt, "xrn": xrn, "bcv": bcv_full})
    return in_maps


def kernel(**inputs):
    x_mic = np.asarray(inputs["x_mic"], np.float32)
    x_ref = np.asarray(inputs["x_ref"], np.float32)
    w_mic = np.asarray(inputs["w_mic"], np.float32)
    b_mic = np.asarray(inputs["b_mic"], np.float32)
    w_ref = np.asarray(inputs["w_ref"], np.float32)
    b_ref = np.asarray(inputs["b_ref"], np.float32)
    w_conv = np.asarray(inputs["w_conv"], np.float32)
    b_conv = np.asarray(inputs["b_conv"], np.float32)
    delay = int(np.asarray(inputs["delay"]))

    if (
        x_mic.shape != (B, C, T, F)
        or x_ref.shape != (B, C, T, F)
        or delay != D
        or w_conv.shape != (1, H, 5, 3)
    ):
        return _np_reference(
            x_mic, x_ref, w_mic, b_mic, w_ref, b_ref, w_conv, b_conv, delay
        )

    from concourse.bass_utils import run_bass_kernel_spmd

    if "nc" not in _CACHE:
        _CACHE["nc"] = _build_graph()
    nc = _CACHE["nc"]

    in_maps = _prepare_inputs(x_mic, x_ref, w_mic, b_mic, w_ref, b_ref, w_conv)
    res = run_bass_kernel_spmd(nc, in_maps, core_ids=list(range(NCORES)))

    y = np.zeros((B, C, TP, F), np.float32)
    for i in range(NCORES):
        y[:, :, i * TL : (i + 1) * TL] = res.results[i]["out"]
    return np.ascontiguousarray(y[:, :, :T]).astype(np.float32)


if __name__ == "__main__":
    rng = np.random.default_rng(0)
    ins = {
        "x_mic": rng.standard_normal((B, C, T, F), np.float32),
        "x_ref": rng.standard_normal((B, C, T, F), np.float32),
        "w_mic": rng.standard_normal((H, C), np.float32) * 0.5,
        "b_mic": rng.standard_normal((H,), np.float32) * 0.1,
        "w_ref": rng.standard_normal((H, C), np.float32) * 0.5,
        "b_ref": rng.standard_normal((H,), np.float32) * 0.1,
        "w_conv": rng.standard_normal((1, H, 5, 3), np.float32) * 0.05,
        "b_conv": rng.standard_normal((1,), np.float32) * 0.1,
        "delay": D,
    }
    got = kernel(**ins)
    want = _np_reference(**ins)
    err = np.linalg.norm(got - want) / np.linalg.norm(want)
    print("rel err vs numpy ref:", err)



# revision 64
# speedup vs baseline: 1.2495x; 1.2495x over previous
"""AlignBlock kernel for 8 TRN2 NeuronCores.

Reference computation (B=2, C=2, T=500, F=129, H=16, D=100):
  Q = conv1x1(x_mic; w_mic, b_mic)        (B,H,T,F)
  K = conv1x1(x_ref; w_ref, b_ref)        (B,H,T,F)
  V[b,h,t,d]  = sum_f Q[b,h,t,f] * Kpad[b,h,t-99+d,f]       (delay window)
  V2 = conv2d(V, w_conv (1,H,5,3), causal-T pad (4,0), d pad (1,1)) + b_conv
  A  = softmax_d(V2[:,0])                 (B,T,D)
  y[b,c,t,f] = sum_d x_refpad[b,c,t-99+d,f] * A[b,t,d]

Key algebraic restructuring (all exact):
  - The H dimension is folded on the host: with augmented channels
    xm~ = [xm0, xm1, u], xr~ = [xr0, xr1, v] (u/v = validity masks emulating
    the reference's zero padding of Q rows / K columns), the conv input
    planes are sum_h w_conv[h]*V[h] = sum_{q=(cm,cr)} Wc[q] * XC[q] where
    XC[q][t,t'] = sum_f xm~[cm,t,f] xr~[cr,t',f]  (9 raw correlation planes)
    and Wc[q,i,j] = sum_h w_conv[h,i,j] wm~[h,cm] wr~[h,cr].
  - The causal 5-tap T conv becomes banded-matrix matmuls (contraction over
    conv input rows); the 3 d-taps are free-dim shifted column reads.
  - softmax(V2 + b_conv) == softmax(V2): b_conv is dropped.
  - y is a matmul contracting t' with the banded attention matrix A_band.

Sharding: sequence-parallel over T, 63 output frames/core (T padded 500->504),
each core loads its input slice with halos host-side; no collectives.

DMA scheduling (measured on TRN2 DGE):
  - strided-DRAM-side DMAs pace ~27-45ns/descriptor serially per queue;
    contiguous flat <=2KB descriptors flow concurrently (~3-13ns each).
  - per-queue sustained throughput caps around 25-60GB/s; the band writes
    and diagonal regathers are row-split/spread across the sync HWDGE,
    scalar HWDGE and gpsimd SWDGE queues.
  - softmax is max-free (logits ~ +-17), normalization is folded into the
    final psum read-out, and the attention skew write is one flat
    aggregating row write per batch feeding a single xbar transpose pair.
"""

import os
import sys

import numpy as np

sys.path.insert(0, "/opt/trn_rl_repo")

# ---- problem constants (hardcoded per the staged problem) ----
B, C, T, F = 2, 2, 500, 129
H, D = 16, 100
NCORES = 8
TL = 63               # output frames per core
TP = NCORES * TL      # padded T = 504
R = TL + 4            # conv input rows per core (67)
TH = TL + D + 3       # x_ref halo columns per core (166)
NQ = 9                # augmented channel pairs
DW = D + 2            # padded delay width incl. zero edge cols (102)

_CACHE = {}


def _np_reference(x_mic, x_ref, w_mic, b_mic, w_ref, b_ref, w_conv, b_conv, delay):
    """Pure-numpy fallback, exact mirror of the jax reference."""
    Bn, Cn, Tn, Fn = x_mic.shape
    Dn = int(delay)
    Q = np.einsum("bctf,hc->bhtf", x_mic, w_mic) + b_mic[None, :, None, None]
    K = np.einsum("bctf,hc->bhtf", x_ref, w_ref) + b_ref[None, :, None, None]
    idx = np.arange(Tn)[:, None] + np.arange(Dn)[None, :]
    Kp = np.pad(K, ((0, 0), (0, 0), (Dn - 1, 0), (0, 0)))
    Ku = Kp[:, :, idx, :]
    V = np.einsum("bhtf,bhtdf->bhtd", Q, Ku)
    Hh = w_conv.shape[1]
    Vp = np.pad(V, ((0, 0), (0, 0), (4, 0), (1, 1)))
    out = np.zeros((Bn, Tn, Dn), np.float32)
    for i in range(5):
        for j in range(3):
            out += np.einsum(
                "bhtd,h->btd", Vp[:, :, i : i + Tn, j : j + Dn], w_conv[0, :, i, j]
            )
    out += b_conv[0]
    m = out.max(-1, keepdims=True)
    e = np.exp(out - m)
    A = e / e.sum(-1, keepdims=True)
    Rp = np.pad(x_ref, ((0, 0), (0, 0), (Dn - 1, 0), (0, 0)))
    Ru = Rp[:, :, idx, :]
    return np.einsum("bctdf,btd->bctf", Ru, A).astype(np.float32)


def _build_graph():
    """Build + compile the single-core SPMD Bass graph (identical on all cores)."""
    from concourse import bacc, mybir, tile

    dt = mybir.dt
    f32 = dt.float32
    bf16 = dt.bfloat16

    nc = bacc.Bacc(
        "TRN2", target_bir_lowering=False, debug=False, num_devices=NCORES
    )

    # external I/O (per-core shards, host-prepared layouts)
    xmt = nc.dram_tensor("xmt", [F, B, 3, R], bf16, kind="ExternalInput")
    xrt = nc.dram_tensor("xrt", [F, B, 3, TH], bf16, kind="ExternalInput")
    xrn = nc.dram_tensor("xrn", [TH, B, C, F], bf16, kind="ExternalInput")
    # banded conv weights + 6 trailing cols holding the f=128 row of xmt as
    # per-partition scalars for the fused rank-1 correlation correction
    NBW = NQ * 3 * TL
    bcv = nc.dram_tensor("bcv", [R, NBW + 6], bf16, kind="ExternalInput")
    out = nc.dram_tensor("out", [B, C, TL, F], f32, kind="ExternalOutput")

    # per-b correlation scratch: 3 cm-planes [g][r][t'*3] (q-triplets
    # interleaved stride-3 within each plane)
    GQ = 3
    sxc = [nc.dram_tensor(f"sxc{b}", [3 * R * TH * GQ + 64], bf16) for b in range(B)]
    # row-banded attention scratch, one tensor per batch (DRAM dependency
    # tracking is tensor-granular; separate tensors let b0's xbar regather
    # start before b1's softmax lands)
    SGS = 64 * 256
    sg = [nc.dram_tensor(f"sg{b}", [SGS + 64], bf16) for b in range(B)]

    VecI64Pair = None

    def strided_ap(handle, offset_el, dims):
        """AP on a flat DRAM tensor with explicit [stride, size] dims."""
        nonlocal VecI64Pair
        a = handle.ap().copy()
        if VecI64Pair is None:
            VecI64Pair = type(a.ap)
        a.ap = VecI64Pair([list(d) for d in dims])
        a.offset = offset_el
        return a

    with tile.TileContext(nc) as tc:
        with (
            tc.tile_pool(name="w", bufs=1) as wp,
            tc.tile_pool(name="xcps", bufs=3, space="PSUM") as xcp,
            tc.tile_pool(name="cvps", bufs=1, space="PSUM") as cvp,
            tc.tile_pool(name="vps", bufs=2, space="PSUM") as vp,
            tc.tile_pool(name="yps", bufs=2, space="PSUM") as yp,
            tc.tile_pool(name="st", bufs=6) as sp,
            tc.tile_pool(name="sm", bufs=2) as smp,
        ):
            # ---- persistent input tiles ----
            xmt0 = wp.tile([128, B, 3, R], bf16, tag="xmt0")
            xmt1 = wp.tile([1, B, 3, R], bf16, tag="xmt1")
            xrt0 = wp.tile([128, B, 3, TH], bf16, tag="xrt0")
            xrt1 = wp.tile([1, B, 3, TH], bf16, tag="xrt1")
            xrn0 = wp.tile([128, B, C, F], bf16, tag="xrn0")
            xrn1 = wp.tile([TH - 128, B, C, F], bf16, tag="xrn1")
            bcw = wp.tile([R, NBW + 6], bf16, tag="bcw")

            # XC-critical tensors first; big flowing loads on sync whose queue
            # the band writes now avoid, tinies + halves on scalar
            nc.sync.dma_start(out=xrt0[:], in_=xrt[0:128])
            nc.scalar.dma_start(out=xrt1[:], in_=xrt[128:129])
            nc.sync.dma_start(out=xmt0[:], in_=xmt[0:128])
            nc.scalar.dma_start(out=xmt1[:], in_=xmt[128:129])
            nc.sync.dma_start(out=bcw[0:34], in_=bcv[0:34])
            nc.scalar.dma_start(out=bcw[34:R], in_=bcv[34:R])
            nc.sync.dma_start(out=xrn0[:], in_=xrn[0:128])
            nc.scalar.dma_start(out=xrn1[:], in_=xrn[128:TH])

            # PE warm-up: dense dummy matmuls during the DMA load prologue to
            # release the HAM clock gate before real matmuls.
            wsrc = wp.tile([128, 512], bf16, tag="wsrc")
            nc.gpsimd.memset(wsrc[:], 0.0)
            wps = cvp.tile([128, 512], f32, tag="cvs")
            for _ in range(6):
                nc.tensor.matmul(
                    out=wps[:], lhsT=wsrc[:, 0:128], rhs=wsrc[:],
                    start=True, stop=True,
                )

            # attention tile [128, 257]: b blocks at 64-partition alignment,
            # exp written to cols [4:104), rest stays zero; full 257-wide rows
            # written flat to sg give the banded (skewed) layout with
            # aggregating descriptors.
            att = wp.tile([128, 257], bf16, tag="att")
            nc.gpsimd.memset(att[:], 0.0)
            # zero the sg tail beyond the last written row in each b block
            ztail = wp.tile([2, SGS - TL * 257], bf16, tag="ztail")
            nc.gpsimd.memset(ztail[:], 0.0)
            nc.gpsimd.dma_start(
                out=strided_ap(sg[0], TL * 257, [[SGS, 1], [1, SGS - TL * 257]]),
                in_=ztail[0:1, :],
            )
            nc.gpsimd.dma_start(
                out=strided_ap(sg[1], TL * 257, [[SGS, 1], [1, SGS - TL * 257]]),
                in_=ztail[1:2, :],
            )

            # rect correlation planes: [r, b, g, dw, qs], edge cols stay zero
            xcd = wp.tile([R, B, 3, DW, GQ], bf16, tag="xcd")
            nc.gpsimd.memset(xcd[:], 0.0)

            # ones row for broadcasting the f=128 xrt row across partitions
            # via a single K=1 matmul per batch (replaces four K=1 matmuls;
            # a K=1 matmul costs as much as the K=128 one)
            ones1 = wp.tile([1, R], bf16, tag="ones1")
            nc.gpsimd.memset(ones1[:], 1.0)
            rep = [None, None]

            # ---- stage 1: XC matmuls -> psum read-out (vector fuses the
            # rank-1 f=128 term; activation engine copies for 2 planes whose
            # K=1 matmul stays) -> band writes ----
            for b in range(B):
                for g in range(3):
                    xsb = sp.tile([R, TH, GQ], bf16, tag="xsb")
                    on_vec = (b, g) in ((0, 0), (0, 1), (0, 2), (1, 2))
                    if on_vec and rep[b] is None:
                        rtile = cvp.tile([R, GQ, TH], f32, tag="cvs")
                        nc.tensor.matmul(
                            out=rtile[:], lhsT=ones1[:],
                            rhs=xrt1[:, b, :, :], start=True, stop=True,
                        )
                        # DVE reads at most one PSUM operand: stage in SBUF
                        rsb = wp.tile([R, GQ, TH], bf16, tag=f"rsb{b}")
                        nc.scalar.activation(
                            out=rsb[:], in_=rtile[:],
                            func=mybir.ActivationFunctionType.Copy,
                        )
                        rep[b] = rsb
                    # one batched matmul computes the 3 (cm, cr) planes
                    pxc = xcp.tile([R, GQ, TH], f32, tag="pxc")
                    nc.tensor.matmul(
                        out=pxc[:], lhsT=xmt0[:, b, g, :],
                        rhs=xrt0[:, b, :, :], start=True, stop=on_vec,
                    )
                    if not on_vec:
                        nc.tensor.matmul(
                            out=pxc[:], lhsT=xmt1[:, b, g, :],
                            rhs=xrt1[:, b, :, :], start=False, stop=True,
                        )
                    xout = xsb[:].transpose([0, 2, 1])
                    if on_vec:
                        nc.vector.scalar_tensor_tensor(
                            out=xout, in0=rep[b][:],
                            scalar=bcw[:, NBW + 3 * b + g : NBW + 3 * b + g + 1],
                            in1=pxc[:],
                            op0=mybir.AluOpType.mult, op1=mybir.AluOpType.add,
                        )
                    else:
                        nc.scalar.activation(
                            out=xout, in_=pxc[:],
                            func=mybir.ActivationFunctionType.Copy,
                        )
                    # contiguous flat DRAM dst; keep the paced sync queue free
                    # for regathers: scalar takes 45 rows, SWDGE the rest
                    base = g * R * TH * GQ
                    for weng, r0, r1 in (
                        (nc.scalar, 0, 45), (nc.gpsimd, 45, R),
                    ):
                        weng.dma_start(
                            out=strided_ap(
                                sxc[b], base + r0 * TH * GQ,
                                [[TH * GQ, r1 - r0], [1, TH * GQ]],
                            ),
                            in_=xsb[r0:r1],
                        )

            # diagonal regather xcd[r, b, g, 1+d, qs] = XC_q[r, r+d]; sync's
            # queue is idle after the loads, so it takes the b0 planes (and
            # one more) while scalar finishes the band writes
            rg_eng = {
                (0, 0): (nc.sync,), (0, 1): (nc.sync,), (0, 2): (nc.sync,),
                (1, 0): (nc.scalar,), (1, 1): (nc.sync, nc.scalar),
                (1, 2): (nc.scalar,),
            }
            for b in range(B):
                for g in range(3):
                    engs = rg_eng[(b, g)]
                    for i, eng in enumerate(engs):
                        r0 = 0 if i == 0 else 34
                        r1 = R if i == len(engs) - 1 else 34
                        eng.dma_start(
                            out=xcd[r0:r1, b, g, 1 : 1 + D, :],
                            in_=strided_ap(
                                sxc[b],
                                g * R * TH * GQ + r0 * GQ * (TH + 1),
                                [[GQ * (TH + 1), r1 - r0], [1, D * GQ]],
                            ),
                        )

            # ---- stage 2: folded conv as banded matmuls, split per batch so
            # b0's softmax/attention tail overlaps b1's conv ----
            n_mm = NQ * 3
            v2b = []
            for b in range(B):
                vb = vp.tile([TL, D], f32, tag="v2")
                k = 0
                for g in range(3):
                    for qs in range(GQ):
                        q = g * GQ + qs
                        for j in range(3):
                            kb = (q * 3 + j) * TL
                            nc.tensor.matmul(
                                out=vb[:],
                                lhsT=bcw[:, kb : kb + TL],
                                rhs=xcd[:, b, g, j : j + D, qs],
                                start=(k == 0), stop=(k == n_mm - 1),
                            )
                            k += 1
                v2b.append(vb)

            # ---- stages 3+4, pipelined per batch: softmax (max-free: logits
            # ~ +-17, exp fits fp32/bf16), banded skew write, xbar regather,
            # delay-weighted matmul with normalization folded into the psum
            # read-out, contiguous per-plane output write ----
            ssum = smp.tile([128, 1], f32, tag="ssum")
            rin = smp.tile([128, 1], f32, tag="rin")
            yout = wp.tile([TL, B, C, F], f32, tag="yout")
            for b in range(B):
                nc.scalar.activation(
                    out=att[64 * b : 64 * b + 63, 4:104], in_=v2b[b][:],
                    func=mybir.ActivationFunctionType.Exp,
                    accum_out=ssum[64 * b : 64 * b + 63, :],
                )
                nc.vector.reciprocal(
                    rin[64 * b : 64 * b + 63, :], ssum[64 * b : 64 * b + 63, :]
                )
                # unnormalized exp -> banded DRAM scratch (flat, aggregating)
                weng = nc.sync if b == 0 else nc.scalar
                weng.dma_start(
                    out=strided_ap(sg[b], 0, [[257, TL], [1, 257]]),
                    in_=att[64 * b : 64 * b + 63, :],
                )
                # per-batch xbar pair, both on one queue so neither instruction
                # gets parked behind the other batch's waits
                xeng = nc.sync if b == 0 else nc.scalar
                a0 = wp.tile([128, 64], bf16, tag=f"a0_{b}")
                xeng.dma_start_transpose(
                    out=a0[:],
                    in_=strided_ap(sg[b], 0, [[256, 64], [1, 128]]),
                )
                a1 = wp.tile([128, 64], bf16, tag=f"a1_{b}")
                xeng.dma_start_transpose(
                    out=a1[:],
                    in_=strided_ap(sg[b], 128, [[256, 64], [1, 128]]),
                )
                for c in range(C):
                    py = yp.tile([TL, F], f32, tag="py")
                    nc.tensor.matmul(
                        out=py[:], lhsT=a0[:, 0:TL], rhs=xrn0[:, b, c, :],
                        start=True, stop=False,
                    )
                    nc.tensor.matmul(
                        out=py[:], lhsT=a1[0 : TH - 128, 0:TL],
                        rhs=xrn1[:, b, c, :],
                        start=False, stop=True,
                    )
                    nc.vector.tensor_scalar_mul(
                        yout[:, b, c, :], py[:], rin[64 * b : 64 * b + 63, :]
                    )
                    eng = nc.sync if c == 0 else nc.scalar
                    eng.dma_start(
                        out=strided_ap(
                            out, (b * C + c) * TL * F, [[F, TL], [1, F]]
                        ),
                        in_=yout[:, b, c, :],
                    )

    nc.compile()
    return nc


def _prepare_inputs(x_mic, x_ref, w_mic, b_mic, w_ref, b_ref, w_conv):
    """Host-side sharding + weight folding. Returns in_maps (one dict/core)."""
    from ml_dtypes import bfloat16

    # padded arrays: xm rows [t0-4, t0+63), xr cols [t0-103, t0+63)
    xm_pad = np.zeros((B, C, 4 + TP, F), np.float32)
    xm_pad[:, :, 4 : 4 + T] = x_mic
    xr_pad = np.zeros((B, C, D + 3 + TP, F), np.float32)
    xr_pad[:, :, D + 3 : D + 3 + T] = x_ref

    # folded conv weights: Wc[cm, cr, i, j] = sum_h w_conv * wm~ * wr~
    wt = np.asarray(w_conv, np.float64)[0]          # (H, 5, 3)
    wtm = np.concatenate([w_mic, b_mic[:, None]], 1).astype(np.float64)  # (H,3)
    wtr = np.concatenate([w_ref, b_ref[:, None]], 1).astype(np.float64)  # (H,3)
    Wc = np.einsum("hij,hm,hr->mrij", wt, wtm, wtr)  # (3,3,5,3)

    # banded conv matrices bcv[r, q, j, tau] = Wc[q, r-tau, j]
    bcv = np.zeros((R, 3, 3, 3, TL), np.float32)
    for i in range(5):
        for j in range(3):
            bcv[np.arange(TL) + i, :, :, j, np.arange(TL)] = np.float32(
                Wc[:, :, i, j]
            )[None]
    bcv = bcv.reshape(R, NQ * 3 * TL).astype(bfloat16)

    in_maps = []
    for i in range(NCORES):
        t0 = i * TL
        xm_s = xm_pad[:, :, t0 : t0 + R]          # (B,C,R,F) rows t0-4..t0+62
        xr_s = xr_pad[:, :, t0 : t0 + TH]         # (B,C,TH,F) cols t0-103..t0+62
        u = (np.arange(R) + t0 - 4 >= 0).astype(np.float32)
        v = (np.arange(TH) + t0 - D - 3 >= 0).astype(np.float32)

        xmt = np.empty((B, 3, R, F), np.float32)
        xmt[:, :C] = xm_s
        xmt[:, C] = u[:, None]
        xmt = np.ascontiguousarray(xmt.transpose(3, 0, 1, 2)).astype(bfloat16)
        # f=128 row of xmt as per-partition scalars, appended to bcv cols
        xm1c = np.ascontiguousarray(xmt[128].transpose(2, 0, 1).reshape(R, 6))
        bcv_full = np.concatenate([bcv, xm1c], axis=1)

        xrt = np.empty((B, 3, TH, F), np.float32)
        xrt[:, :C] = xr_s
        xrt[:, C] = v[:, None]
        xrt = np.ascontiguousarray(xrt.transpose(3, 0, 1, 2)).astype(bfloat16)

        xrn = np.ascontiguousarray(xr_s.transpose(2, 0, 1, 3)).astype(bfloat16)

        in_maps.append({"xmt": xmt, "xrt": xrt, "xrn": xrn, "bcv": bcv_full})
    return in_maps


def kernel(**inputs):
    x_mic = np.asarray(inputs["x_mic"], np.float32)
    x_ref = np.asarray(inputs["x_ref"], np.float32)
    w_mic = np.asarray(inputs["w_mic"], np.float32)
    b_mic = np.asarray(inputs["b_mic"], np.float32)
    w_ref = np.asarray(inputs["w_ref"], np.float32)
    b_ref = np.asarray(inputs["b_ref"], np.float32)
    w_conv = np.asarray(inputs["w_conv"], np.float32)
    b_conv = np.asarray(inputs["b_conv"], np.float32)
    delay = int(np.asarray(inputs["delay"]))

    if (
        x_mic.shape != (B, C, T, F)
        or x_ref.shape != (B, C, T, F)
        or delay != D
        or w_conv.shape != (1, H, 5, 3)
    ):
        return _np_reference(
            x_mic, x_ref, w_mic, b_mic, w_ref, b_ref, w_conv, b_conv, delay
        )

    from concourse.bass_utils import run_bass_kernel_spmd

    if "nc" not in _CACHE:
        _CACHE["nc"] = _build_graph()
    nc = _CACHE["nc"]

    in_maps = _prepare_inputs(x_mic, x_ref, w_mic, b_mic, w_ref, b_ref, w_conv)
    res = run_bass_kernel_spmd(nc, in_maps, core_ids=list(range(NCORES)))

    y = np.zeros((B, C, TP, F), np.float32)
    for i in range(NCORES):
        y[:, :, i * TL : (i + 1) * TL] = res.results[i]["out"]
    return np.ascontiguousarray(y[:, :, :T]).astype(np.float32)


if __name__ == "__main__":
    rng = np.random.default_rng(0)
    ins = {
        "x_mic": rng.standard_normal((B, C, T, F), np.float32),
        "x_ref": rng.standard_normal((B, C, T, F), np.float32),
        "w_mic": rng.standard_normal((H, C), np.float32) * 0.5,
        "b_mic": rng.standard_normal((H,), np.float32) * 0.1,
        "w_ref": rng.standard_normal((H, C), np.float32) * 0.5,
        "b_ref": rng.standard_normal((H,), np.float32) * 0.1,
        "w_conv": rng.standard_normal((1, H, 5, 3), np.float32) * 0.05,
        "b_conv": rng.standard_normal((1,), np.float32) * 0.1,
        "delay": D,
    }
    got = kernel(**ins)
    want = _np_reference(**ins)
    err = np.linalg.norm(got - want) / np.linalg.norm(want)
    print("rel err vs numpy ref:", err)

